# revision 28
# baseline (speedup 1.0000x reference)
"""Trainium2 Bass kernel for nn_MultiHeadAttention_5334349382389.

Sharding: 8 cores = 4 batches x 2 head-groups (4 heads each).
Core c handles batch b = c // 2, head-group g = c % 2 (heads 4g..4g+3).

v2 design (vs baseline): ACT(exp)-saturated mid-section.
  - Ordered input DMA (weights -> xq -> xk -> xv -> wo -> edge last) so
    projections start ~1us in instead of waiting ~20us for all inputs.
  - Scores via K=64 row-tiled matmuls: even head of a pair at PE rows
    0-63, odd head at rows 64-127 -> the two matmuls run concurrently
    (no zero-padded K=128 waste).
  - Pair-interleaved exp: head-even / head-odd score tiles alternate, so
    while ACT reads one head's scores the PE refills the other's. ACT
    stays ~100% busy. PSUM: 4 score banks + 4 attnV accum banks = 8.
  - Heads (2,3) processed first, (0,1) last so the edge matrix DMA (only
    needed by head 0) has ~25us to land.
  - attnV uses the ones-column trick (M=65) for softmax denominators.
  - exp table preloaded at t=0; junk matmuls only as initial HAM warmup.

Host: packs/transposes/casts inputs per core, gathers
  out[b] = partial(b,0) + partial(b,1) + (bo + Wo @ bv)
(the bv term is exact because softmax rows sum to 1).
"""

import os
import sys

sys.path.insert(0, "/opt/trn_rl_repo")

import numpy as np

B, SEQ, DIN, DO = 4, 1024, 512, 512
NH_ALL, DK = 8, 64
NHC = 4            # heads per core
DH = NHC * DK      # 256 per-core projected dims
P = 128
CD = DIN // P      # 4 contraction chunks for projections
CH = DH // P       # 2 dh chunks (head pairs)
KT = SEQ // P      # 8 k-tiles
STR = 512          # q-stripe (matmul free dim)
NS = SEQ // STR    # 2 stripes
TVW = NHC * (DK + 1) + DK - 1  # 323: per-k-tile v-aux width (4x65 + 63 pad)
                               # windows are 128 wide so attnV matmuls keep the
                               # full PE array active (HAM stays at 2.4 GHz)

# packed weight tensor: wq | wk | wv | ident | wo
NW = P + CD * DH + CD * DH + CD * DH + CH * DO   # 128 + 3*1024 + 1024 = 4224
NX = CD * SEQ                                    # 4096 per x tensor

COMPUTE = os.environ.get("KERNEL_COMPUTE_DT", "fp16")  # fp16 | bf16

_nc = None


def _np_dt():
    import ml_dtypes

    return {
        "fp16": np.float16,
        "bf16": ml_dtypes.bfloat16,
    }[COMPUTE]


def _build():
    global _nc
    if _nc is not None:
        return _nc
    import concourse.bacc as bacc
    import concourse.bass as bass
    import concourse.mybir as mybir
    import concourse.tile as tile

    f32 = mybir.dt.float32
    cdt = {
        "fp16": mybir.dt.float16,
        "bf16": mybir.dt.bfloat16,
    }[COMPUTE]
    Exp = mybir.ActivationFunctionType.Exp

    nc = bacc.Bacc("TRN2", target_bir_lowering=False, debug=False)

    pk_w = nc.dram_tensor("pk_w", (P, NW), cdt, kind="ExternalInput")
    pk_xq = nc.dram_tensor("pk_xq", (P, NX), cdt, kind="ExternalInput")
    pk_xk = nc.dram_tensor("pk_xk", (P, NX), cdt, kind="ExternalInput")
    pk_xv = nc.dram_tensor("pk_xv", (P, NX), cdt, kind="ExternalInput")
    bqk = nc.dram_tensor("bqk", (2 * DH, 1), f32, kind="ExternalInput")
    edge = nc.dram_tensor("edge", (SEQ, SEQ), cdt, kind="ExternalInput")
    outp = nc.dram_tensor("outp", (SEQ, DO), cdt, kind="ExternalOutput")

    edge_r = edge.rearrange("(t p) n -> t p n", p=P)
    out_r = outp.rearrange("(t p) n -> p t n", p=P)

    def sl(s):
        return slice(s * STR, (s + 1) * STR)

    W_WQ = 0
    W_WK = W_WQ + CD * DH
    W_WV = W_WK + CD * DH
    W_EYE = W_WV + CD * DH
    W_WO = W_EYE + P

    with tile.TileContext(nc) as tc:
        with (
            tc.tile_pool(name="inp", bufs=1) as inp,
            tc.tile_pool(name="wts", bufs=1) as wts,
            tc.tile_pool(name="qkp", bufs=1) as qkp,
            tc.tile_pool(name="vhap", bufs=1) as vhap,
            tc.tile_pool(name="expp", bufs=8) as expp,
            tc.tile_pool(name="otp", bufs=1) as otp,
            tc.tile_pool(name="rrp", bufs=4) as rrp,
            tc.tile_pool(name="rbp", bufs=2) as rbp,
            tc.tile_pool(name="outsp", bufs=1) as outsp,
            tc.tile_pool(name="edgp", bufs=1) as edgp,
            # PSUM: spp = 2x [128,1024] f32 (4 banks), accp = 4x [128,512] (4 banks)
            tc.tile_pool(name="spp", bufs=2, space=bass.MemorySpace.PSUM) as spp,
            tc.tile_pool(name="accp", bufs=4, space=bass.MemorySpace.PSUM) as accp,
        ):
            # ---------------- input tiles ----------------
            tw = inp.tile([P, NW], cdt, tag="tw")
            txq = inp.tile([P, CD, SEQ], cdt, tag="txq")
            txk = inp.tile([P, CD, SEQ], cdt, tag="txk")
            txv = inp.tile([P, CD, SEQ], cdt, tag="txv")
            tb4 = wts.tile([P, 4, 1], f32, tag="tb4")

            # junk weights from memset first: PE warmup needs no DMA at all
            jw = wts.tile([P, P], cdt, tag="jw")
            nc.gpsimd.memset(jw[:], 0.125)

            # Each transfer streams on ONE hw queue at ~110GB/s; concurrency
            # comes from spreading transfers across the 3 DMA-capable rings
            # (sync/SP, scalar/ACT, gpsimd) in lockstep thirds, ordered by
            # first use: wq,wk -> xq -> xk -> wv -> xv -> ident+wo -> edge.
            nc.scalar.dma_start(out=tb4, in_=bqk.rearrange("(c p) o -> p c o", p=P))
            txqf = txq.rearrange("p c n -> p (c n)")
            txkf = txk.rearrange("p c n -> p (c n)")
            txvf = txv.rearrange("p c n -> p (c n)")

            def split3(dst, src, rings):
                bounds = [0, NX // 3, 2 * NX // 3, NX]
                for i, ring in enumerate(rings):
                    a, b = bounds[i], bounds[i + 1]
                    ring.dma_start(out=dst[:, a:b], in_=src[:, a:b])

            nc.sync.dma_start(out=tw[:, W_WQ : W_WK], in_=pk_w[:, W_WQ : W_WK])
            nc.scalar.dma_start(out=tw[:, W_WK : W_WV], in_=pk_w[:, W_WK : W_WV])
            split3(txqf, pk_xq, (nc.sync, nc.scalar, nc.gpsimd))
            split3(txkf, pk_xk, (nc.sync, nc.scalar, nc.gpsimd))
            nc.gpsimd.dma_start(out=tw[:, W_WV : W_EYE], in_=pk_w[:, W_WV : W_EYE])
            split3(txvf, pk_xv, (nc.sync, nc.scalar, nc.gpsimd))
            nc.scalar.dma_start(out=tw[:, W_EYE : NW], in_=pk_w[:, W_EYE : NW])

            teye = tw[:, W_EYE : W_EYE + P]
            twq = tw[:, W_WQ : W_WK].rearrange("p (c d) -> p c d", d=DH)
            twk = tw[:, W_WK : W_WV].rearrange("p (c d) -> p c d", d=DH)
            twv = tw[:, W_WV : W_EYE].rearrange("p (c d) -> p c d", d=DH)
            two = tw[:, W_WO : NW].rearrange("p (c d) -> p c d", d=DO)

            # edge: thirds queued last; needed only by pair A (~35us in)
            edt = edgp.tile([P, KT, SEQ], cdt, tag="edg")
            edge_pt = edge.rearrange("(t p) n -> p t n", p=P)
            for i, ring in enumerate((nc.sync, nc.gpsimd, nc.scalar)):
                a = i * KT // 3 if i else 0
                a, b = [0, 3, 6, 8][i], [0, 3, 6, 8][i + 1]
                ring.dma_start(out=edt[:, a:b, :], in_=edge_pt[:, a:b, :])
            eds = [edt[:, kt, :] for kt in range(KT)]

            # v-aux ones columns + zero tail pad (device-side init; no DMA)
            tvha = vhap.tile([P, KT, TVW], cdt, tag="tvha")
            for h in range(NHC):
                nc.gpsimd.memset(tvha[:, :, h * (DK + 1) + DK : h * (DK + 1) + DK + 1], 1.0)
            nc.gpsimd.memset(tvha[:, :, NHC * (DK + 1) : TVW], 0.0)

            # preload the exp table set (~2.7us) while DMAs land; also HAM warmup
            wrm = expp.tile([P, SEQ], cdt, tag="expT")
            nc.scalar.activation(out=wrm[:, 0:P], in_=jw[:], func=Exp)
            jnk = accp.tile([P, STR], f32, tag="acc")

            def junk(n):
                for _ in range(n):
                    nc.tensor.matmul(
                        jnk[:, 0:P], lhsT=jw[:], rhs=jw[:], start=True, stop=True
                    )

            junk(40)



            # ---------------- projections ----------------
            # tqh per ch: head 2ch at partitions 0-63, head 2ch+1 at 64-127.
            # khp per head slot: kh at the head's 64-partition half, zeros in
            # the other half, so K=128 score matmuls keep the full PE array
            # active (HAM) while masking the other head.
            tqh = qkp.tile([P, CH, SEQ], cdt, tag="tqh")
            khp = qkp.tile([P, NHC, SEQ], cdt, tag="khp")
            nc.gpsimd.memset(khp[0:DK, 1::2, :], 0.0)
            nc.gpsimd.memset(khp[DK:P, 0::2, :], 0.0)

            def proj_q(ch):
                pt = spp.tile([P, SEQ], f32, tag="sc")
                for cd in range(CD):
                    for s in range(NS):
                        nc.tensor.matmul(
                            pt[:, sl(s)],
                            lhsT=twq[:, cd, ch * P : (ch + 1) * P],
                            rhs=txq[:, cd, sl(s)],
                            start=(cd == 0),
                            stop=(cd == CD - 1),
                        )
                nc.vector.tensor_scalar_add(
                    out=tqh[:, ch, :], in0=pt[:], scalar1=tb4[:, ch, :]
                )

            def proj_k(ch):
                pt = spp.tile([P, SEQ], f32, tag="sc")
                for cd in range(CD):
                    for s in range(NS):
                        nc.tensor.matmul(
                            pt[:, sl(s)],
                            lhsT=twk[:, cd, ch * P : (ch + 1) * P],
                            rhs=txk[:, cd, sl(s)],
                            start=(cd == 0),
                            stop=(cd == CD - 1),
                        )
                nc.vector.tensor_scalar_add(
                    out=khp[0:DK, 2 * ch, :],
                    in0=pt[0:DK, :],
                    scalar1=tb4[0:DK, 2 + ch, :],
                )
                nc.vector.tensor_scalar_add(
                    out=khp[DK:P, 2 * ch + 1, :],
                    in0=pt[DK:P, :],
                    scalar1=tb4[DK:P, 2 + ch, :],
                )

            # v: [s, dh] tiles written into vh_aug (65-wide per head, col 64 = 1)
            # v-proj borrows the target PSUM score tile right before the score
            # matmuls clear it (start=True), so it needs no extra PSUM bank.
            def proj_v_into(stt, st):
                for cd in range(CD):
                    nc.tensor.matmul(
                        stt[:, 0:DH],
                        lhsT=txv[:, cd, st * P : (st + 1) * P],
                        rhs=twv[:, cd, :],
                        start=(cd == 0),
                        stop=(cd == CD - 1),
                    )
                nc.vector.tensor_copy(
                    out=tvha[:, st, 0 : NHC * (DK + 1)].rearrange(
                        "p (h w) -> p h w", w=DK + 1
                    )[:, :, 0:DK],
                    in_=stt[:, 0:DH].rearrange("p (h d) -> p h d", h=NHC),
                )

            # q1/q0 gated on xq DMA, k1/k0 on xk (pair B needs only q1+k1)
            proj_q(1)
            proj_q(0)
            proj_k(1)
            proj_k(0)

            # ---------------- attention, one head-pair at a time ----------------
            tot = otp.tile([P, CH, SEQ], cdt, tag="tot")

            def pair_body(ch, lag, carry_in=()):
                # heads: he = 2ch (partitions 0-63), ho = 2ch+1 (64-127)
                # Software-pipelined so ACT never starves: PE issue order is
                #   aV_he(kt-lag), [v-weave], S_he(kt), aV_ho(kt-lag), S_ho(kt)
                # and ACT order is exp_he(kt), exp_ho(kt) -- while one head's
                # exp runs, the other head's score tile is refilled.
                # Pair B: lag=3 + v-projection weave (v(st) borrows the ho
                # score tile at slot st+2, before the scores clear it).
                # Pair A: lag=2 + edge injection + pair B's carried attnVs.
                # attnVs left over at the end are returned as thunks (carry)
                # so the next phase can absorb them in its PE slack.
                is_edge_pair = ch == 0
                weave_v = ch == 1
                pv = {}
                for j in range(2):
                    for s in range(NS):
                        pv[(j, s)] = accp.tile(
                            [P, STR], f32, tag="acc", name=f"pv{ch}_{j}_{s}"
                        )

                def scores_exp(kt, j, stt):
                    h = 2 * ch + j
                    inject = is_edge_pair and j == 0
                    for s in range(NS):
                        nc.tensor.matmul(
                            stt[:, sl(s)],
                            lhsT=khp[:, h, kt * P : (kt + 1) * P],
                            rhs=tqh[:, ch, sl(s)],
                            start=True,
                            stop=not inject,
                        )
                        if inject:
                            nc.tensor.matmul(
                                stt[:, sl(s)],
                                lhsT=teye[:],
                                rhs=eds[kt][:, sl(s)],
                                start=False,
                                stop=True,
                            )
                    te = expp.tile([P, SEQ], cdt, tag="expT")
                    nc.scalar.activation(out=te, in_=stt[:], func=Exp)
                    return te

                def attnv(kt, j, te):
                    h = 2 * ch + j
                    for s in range(NS):
                        nc.tensor.matmul(
                            pv[(j, s)][:, :],
                            lhsT=tvha[:, kt, h * (DK + 1) : h * (DK + 1) + P],
                            rhs=te[:, sl(s)],
                            start=(kt == 0),
                            stop=(kt == KT - 1),
                        )

                if weave_v:
                    # last two v-tiles up front (their weave slots don't exist)
                    for st in (KT - 2, KT - 1):
                        stt = spp.tile([P, SEQ], f32, tag="sc")
                        proj_v_into(stt, st)

                tes = {}
                carry_in = list(carry_in)
                for kt in range(KT):
                    for j in range(2):
                        # absorb carried thunks from the previous pair early
                        if kt < 2:
                            for _ in range(2):
                                if carry_in:
                                    carry_in.pop(0)()
                        stt = spp.tile([P, SEQ], f32, tag="sc")
                        if weave_v and j == 1 and 0 <= kt - 2 < KT - 2:
                            proj_v_into(stt, kt - 2)
                        if kt >= lag:
                            attnv(kt - lag, j, tes.pop((kt - lag, j)))
                        tes[(kt, j)] = scores_exp(kt, j, stt)
                carry = []
                for kt in range(KT - lag, KT):
                    for j in range(2):
                        te = tes.pop((kt, j))
                        carry.append(lambda kt=kt, j=j, te=te: attnv(kt, j, te))

                def normalize():
                    _normalize(ch, pv)

                return carry, normalize

            def _normalize(ch, pv):
                # normalize: tot[j*64:(j+1)*64, ch, :] = pv[0:64] / pv[64]
                for j in range(2):
                    rr = rrp.tile([1, SEQ], f32, tag="rr")
                    rs = rrp.tile([1, SEQ], f32, tag="rs")
                    for s in range(NS):
                        nc.vector.tensor_copy(
                            out=rs[:, sl(s)], in_=pv[(j, s)][DK : DK + 1, :]
                        )
                    nc.vector.reciprocal_approx_fast(out=rr[:], in_=rs[:])
                    rb = rbp.tile([DK, SEQ], f32, tag="rb")
                    nc.gpsimd.partition_broadcast(rb[:], rr[:])
                    for s in range(NS):
                        nc.vector.tensor_mul(
                            tot[j * DK : (j + 1) * DK, ch, sl(s)],
                            pv[(j, s)][0:DK, :],
                            rb[:, sl(s)],
                        )

            carry_b, norm_b = pair_body(1, lag=3)
            carry_a, norm_a = pair_body(0, lag=2, carry_in=carry_b + [norm_b])
            for thunk in carry_a:
                thunk()
            norm_a()

            # ---------------- output projection ----------------
            oall = outsp.tile([P, KT, DO], cdt, tag="oall")
            for m in range(KT):
                po = accp.tile([P, STR], f32, tag="acc")
                for i, ch in enumerate((0, 1)):
                    nc.tensor.matmul(
                        po[:, 0:DO],
                        lhsT=tot[:, ch, m * P : (m + 1) * P],
                        rhs=two[:, ch, :],
                        start=(i == 0),
                        stop=(i == CH - 1),
                    )
                nc.vector.tensor_copy(out=oall[:, m, :], in_=po[:, 0:DO])
                if m % 2 == 1:
                    nc.gpsimd.dma_start(
                        out=out_r[:, m - 1 : m + 1, :], in_=oall[:, m - 1 : m + 1, :]
                    )

    nc.compile()
    _nc = nc
    return nc


def _in_maps(q, k, v, edge_matrix, Wq, bq, Wk, bk, Wv, Wo):
    dt = _np_dt()
    zeros_edge = np.zeros((SEQ, SEQ), dt)
    edge_t = np.ascontiguousarray(edge_matrix.T).astype(dt)
    ident = np.eye(P, dtype=dt)

    def re_cp(m):
        # [C*P, D] -> [P, C*D] (partition-major packing of "(c p) d -> p c d")
        cp, d = m.shape
        return np.ascontiguousarray(
            m.reshape(cp // P, P, d).transpose(1, 0, 2).reshape(P, -1)
        )

    xt = {}
    for b in range(B):
        xt[b] = (
            re_cp(np.ascontiguousarray(q[b].T).astype(dt)),
            re_cp(np.ascontiguousarray(k[b].T).astype(dt)),
            re_cp(np.ascontiguousarray(v[b].T).astype(dt)),
        )
    maps = []
    for c in range(8):
        b, g = c // 2, c % 2
        is_edge = g == 0 and b < 2
        rows = slice(g * DH, (g + 1) * DH)
        wq_c = np.ascontiguousarray(Wq[rows].T) * np.float32(1.0 / 8.0)
        bq_c = (bq[rows] * np.float32(1.0 / 8.0)).copy()
        if is_edge:
            wq_c[:, 0:DK] = 0.0
            bq_c[0:DK] = 0.0
        pkw = np.concatenate(
            [
                re_cp(wq_c.astype(dt)),
                re_cp(np.ascontiguousarray(Wk[rows].T).astype(dt)),
                re_cp(np.ascontiguousarray(Wv[rows].T).astype(dt)),
                ident,
                re_cp(np.ascontiguousarray(Wo[:, rows].T).astype(dt)),
            ],
            axis=1,
        )
        assert pkw.shape == (P, NW)
        maps.append(
            {
                "pk_w": np.ascontiguousarray(pkw),
                "pk_xq": xt[b][0],
                "pk_xk": xt[b][1],
                "pk_xv": xt[b][2],
                "bqk": np.concatenate([bq_c, bk[rows]]).reshape(2 * DH, 1),
                "edge": edge_t if is_edge else zeros_edge,
            }
        )
    return maps


def _ensure_ntff_hook():
    """Register the axon NTFF profile hook if the image's antenv lacks it."""
    import contextlib
    import ctypes
    import types

    try:
        from antenv.axon_hooks import get_axon_ntff_profile_hook  # noqa: F401
        return
    except ImportError:
        pass

    so_path = "/opt/axon/libaxon_pjrt.so"
    try:
        lib = ctypes.CDLL(so_path)
    except OSError:
        return
    if not hasattr(lib, "axon_start_nrt_profile"):
        return
    lib.axon_start_nrt_profile.argtypes = [
        ctypes.POINTER(ctypes.c_int64),
        ctypes.c_size_t,
    ]
    lib.axon_start_nrt_profile.restype = ctypes.c_int64
    lib.axon_stop_nrt_profile.argtypes = [ctypes.c_char_p]
    lib.axon_stop_nrt_profile.restype = ctypes.c_int64

    @contextlib.contextmanager
    def _hook(output_dir, device_ids):
        import jax

        jax.devices()
        if device_ids:
            ids = (ctypes.c_int64 * len(device_ids))(*device_ids)
            rc = lib.axon_start_nrt_profile(ids, len(device_ids))
        else:
            rc = lib.axon_start_nrt_profile(None, 0)
        if rc != 0:
            raise RuntimeError(f"axon_start_nrt_profile rc={rc}")
        try:
            yield
        finally:
            n = lib.axon_stop_nrt_profile(str(output_dir).encode())
            if n < 0:
                raise RuntimeError(f"axon_stop_nrt_profile rc={n}")

    _state = {"hook": _hook}
    mod = types.ModuleType("antenv.axon_hooks")
    mod.get_axon_ntff_profile_hook = lambda: _state["hook"]
    mod.set_axon_ntff_profile_hook = lambda h: _state.__setitem__("hook", h)
    import antenv

    antenv.axon_hooks = mod
    sys.modules["antenv.axon_hooks"] = mod


def kernel(q, k, v, edge_matrix, Wq, bq, Wk, bk, Wv, bv, Wo, bo, _trace=False):
    from concourse.bass_utils import run_bass_kernel_spmd

    if _trace:
        _ensure_ntff_hook()

    q, k, v = (np.asarray(t, np.float32) for t in (q, k, v))
    edge_matrix = np.asarray(edge_matrix, np.float32)
    Wq, bq, Wk, bk, Wv, bv, Wo, bo = (
        np.asarray(t, np.float32) for t in (Wq, bq, Wk, bk, Wv, bv, Wo, bo)
    )

    nc = _build()
    maps = _in_maps(q, k, v, edge_matrix, Wq, bq, Wk, bk, Wv, Wo)
    res = run_bass_kernel_spmd(nc, maps, core_ids=list(range(8)), trace=_trace)

    bo_eff = bo + Wo @ bv
    out = np.empty((B, SEQ, DO), np.float32)
    for b in range(B):
        out[b] = (
            res.results[2 * b]["outp"].astype(np.float32)
            + res.results[2 * b + 1]["outp"].astype(np.float32)
            + bo_eff
        )
    if _trace:
        return out, res
    return out
